# revision 13
# baseline (speedup 1.0000x reference)
# Trainium2 Bass kernel: 3-level inverse 2D Haar DWT (DWTInverse, db1, mode=zero).
#
# Math: for a 2-tap synthesis pair (g0=[u0,u1], g1=[v0,v1]) the transposed convs
# have stride 2 and no overlap, so each level is an independent 2x2 butterfly:
#   out[2i+a, 2j+b] = ga[?]... concretely with Haar (u0=u1=v0=a, v1=-a):
#   out[2i,2j]     = 0.5*(ll+lh+hl+hh)
#   out[2i,2j+1]   = 0.5*(ll+lh-hl-hh)
#   out[2i+1,2j]   = 0.5*(ll-lh+hl-hh)
#   out[2i+1,2j+1] = 0.5*(ll-lh-hl+hh)
# Shapes (64->128->256->512 with matching yh sizes) never trigger the crop branch.
#
# Sharding: pure data parallel over batch N=8 -> core k processes n=k
# (32 channels x full spatial). No cross-core communication.
#
# Layout per core: SBUF partition p = (c, b) = c*4+b, c in [0,32) channels,
# b in [0,4) row-blocks of each image. Row-blocks are butterfly-invariant
# (input rows of block b map to output rows of block b), so every level is
# purely free-dim work with strided writes doing the 2x2 spatial interleave
# for free. Intermediates stay in SBUF (llB resident; level-B output in a
# 2-deep sliding segment ring).
#
# fp16 everywhere at the HBM edge (rel-err gate is 2e-2; fp16 end-to-end
# measures ~7e-4): kernel() casts inputs to fp16 host-side, the device
# writes an fp16 output, host upcasts. Halves HBM traffic to 16MB in +
# 16MB out per core. Measured per-rep steady state (paired device-resident
# slope, R=65): full ~76us, DMA-only ~73us (= 32MB / 438GB/s, exactly the
# 16-port SBUF AXI fabric limit shared by both HWDGE rings), compute-only
# ~77us (DVE-bound: 84 stage-2 stt ops). Both rooflines met and fully
# overlapped; fp32 baseline measured ~185us with the same method.
#
# Scale folding: intermediates carry sigma*true (llA 1/8, llB 1/4, seg 1/2).
# Stage-1 (height pass) runs on TensorE as diagonal-weight matmuls into
# PSUM (sigma folded into the fp16 weights; exact powers of 1/2 for Haar);
# C/D (hi-branch height pass) on GpSimd; stage-2 (width pass) is 4 fused
# scale+add stt ops on DVE with the PSUM operand and strided fp16 writes.
# GpSimd cannot read PSUM, so stage-2 is DVE-only by construction; measured
# alternatives (vector-only butterfly with GpSimd interleave writes, ACT
# prescale, cd_engine=vv, batch loads/stores) were all slower on HW.

import numpy as np
from contextlib import ExitStack

C_PER_CORE = 32
N_CORES = 8

_cache = {}

# Tuning knobs (read at build time; bench scripts override before building).
TUNE = {
    "yh_bufs": 5,
    "out_bufs": 4,
    "seg_bufs": 2,
    "cd_bufs": 3,
    "stage1": "pe",  # "dve" (vector butterfly) | "pe" (height pass on TensorE)
    "batch_loads": False,  # 2 compute-chunks per load DMA (~1MB/channel)
    "batch_stores": False,  # 2 compute-chunks per store DMA (4MB)
    "probe": None,  # None | "dma" (transfers only) | "compute" (engines only)
    "io_dtype": "float16",  # HBM/SBUF dtype; "float32" | "float16" (host casts)
    # Engine assignment (dve path): "v"=DVE, "g"=GpSimd.
    "prescale_engine": "v",  # yh tile sigma prescale: "v" (TS 4x) | "a" (ACT)
    "ab_engine": "vv",  # stage-1 A,B (dense TT, 2x on DVE for fp16)
    "cd_engine": "gg",  # stage-1 C,D (dense TT; GpSimd frees DVE for stage-2)
    "strided_engines": "gggg",  # E,F,G,H interleave writes (1x everywhere)
}


def _build_program(u0, u1, v0, v1, reps=1):
    import concourse.bacc as bacc
    import concourse.mybir as mybir
    import concourse.tile as tile

    f32 = mybir.dt.float32
    fio = getattr(mybir.dt, TUNE["io_dtype"])
    np_fio = mybir.dt.np(fio)
    mult = mybir.AluOpType.mult
    add = mybir.AluOpType.add

    rA = v0 / u0  # +1 for Haar (even taps)
    rB = v1 / u1  # -1 for Haar (odd taps)

    # Bacc (not bass.Bass): its compile() runs generate_event_semaphores,
    # which splits multi-sem waits — TPB instructions can carry only ONE
    # sync wait, and walrus codegen hard-errors otherwise.
    nc = bacc.Bacc(
        "TRN2",
        target_bir_lowering=False,
        debug=False,
        enable_asserts=False,
        num_devices=N_CORES,
    )
    yl_t = nc.dram_tensor("yl", [C_PER_CORE, 64, 64], fio, kind="ExternalInput")
    yh0_t = nc.dram_tensor("yh0", [C_PER_CORE, 3, 256, 256], fio, kind="ExternalInput")
    yh1_t = nc.dram_tensor("yh1", [C_PER_CORE, 3, 128, 128], fio, kind="ExternalInput")
    yh2_t = nc.dram_tensor("yh2", [C_PER_CORE, 3, 64, 64], fio, kind="ExternalInput")
    out_t = nc.dram_tensor("out", [C_PER_CORE, 512, 512], fio, kind="ExternalOutput")

    assert abs(rA - 1.0) < 1e-6 and abs(rB + 1.0) < 1e-6
    sub = mybir.AluOpType.subtract
    use_pe = TUNE["stage1"] == "pe"
    sigmas = [float(u0**6), float(u0**4), float(u0**2)]
    if use_pe:
        # Diagonal weight matrices for the PE height pass: identity plus
        # +/- sigma*I per level, embedded in the NEFF as Const tensors.
        # (sigma = powers of 1/2 for Haar -> exact in fp16.)
        w_dram = {"id": nc.inline_tensor(np.eye(128, dtype=np_fio), "w_id")}
        for si, s in enumerate(sigmas):
            for sgn in (1.0, -1.0):
                w_dram[(s, sgn)] = nc.inline_tensor(
                    (sgn * s * np.eye(128)).astype(np_fio),
                    f"w_{si}_{'p' if sgn > 0 else 'n'}",
                )

    with ExitStack() as ctx:
        tc = ctx.enter_context(tile.TileContext(nc))
        res = ctx.enter_context(tc.tile_pool(name="res", bufs=1))
        if use_pe:
            psum = ctx.enter_context(
                tc.tile_pool(name="psum", bufs=2, space="PSUM")
            )
            w_sb = {}
            for key, dh in w_dram.items():
                wt = res.tile(
                    [128, 128], fio, name="wt", tag=f"w{len(w_sb)}"
                )
                nc.sync.dma_start(out=wt[:, :], in_=dh[:, :])
                w_sb[key] = wt
        yh_pool = ctx.enter_context(tc.tile_pool(name="yh", bufs=TUNE["yh_bufs"]))
        abcd = ctx.enter_context(tc.tile_pool(name="abcd", bufs=TUNE["cd_bufs"]))
        outp = ctx.enter_context(tc.tile_pool(name="outp", bufs=TUNE["out_bufs"]))
        segp = ctx.enter_context(tc.tile_pool(name="segp", bufs=TUNE["seg_bufs"]))

        # Resident ll tiles (per partition: rows of my block, dense row-major).
        llA = res.tile([128, 16 * 64], fio, name="llA")     # 0.125 * yl
        llB = res.tile([128, 32 * 128], fio, name="llB")    # 0.25 * level-A out
        # level-B output (0.5 * ll_C) lives in a sliding ring of 2 segments
        # of 16 rows x 256 (llC_seg) so level C can start while level B is
        # still producing, and the freed SBUF buys deeper yh prefetch.

        yh0_v = yh0_t[:, :, :, :].rearrange("c k (b r) w -> c k b r w", b=4)
        yh1_v = yh1_t[:, :, :, :].rearrange("c k (b r) w -> c k b r w", b=4)
        yh2_v = yh2_t[:, :, :, :].rearrange("c k (b r) w -> c k b r w", b=4)
        out_v = out_t[:, :, :].rearrange("c (b r) w -> (c b) r w", b=4)

        def load_yh(yh_v, W, RL, r0, sigma):
            """Load RL rows x W of all 3 detail channels in one 3-dim DMA
            per channel and pre-scale the whole tile by sigma on ACT (one
            ACT op; it also funnels the 3 DMA semaphores into one ACT
            semaphore for all downstream consumers). Returns [p,k,r,w]."""
            probe = TUNE["probe"]
            yh_tile = yh_pool.tile([128, 3 * RL * W], fio, name="yh_tile", tag="yh")
            yh3 = yh_tile.rearrange("p (k r w) -> p k r w", k=3, r=RL)
            if probe != "compute":
                for k in range(3):
                    nc.sync.dma_start(
                        out=yh3[:, k], in_=yh_v[:, k, :, r0 : r0 + RL, :]
                    )
            else:
                # probe tiles need a writer to get a slot allocated
                nc.vector.memset(yh_tile[0:1, 0:1], 0.0)
            if probe != "dma" and not use_pe:
                # DVE stage1 wants pre-scaled details; the PE path folds
                # sigma into the matmul weights / stt scalars instead.
                # TS on DVE hits 4x perf mode for dense fp16 (0.26 ns/elem).
                if TUNE["prescale_engine"] == "v":
                    nc.vector.tensor_scalar(
                        yh_tile[:, :], yh_tile[:, :], float(sigma), None, mult
                    )
                else:
                    nc.scalar.mul(yh_tile[:, :], yh_tile[:, :], float(sigma))
            return yh3

        def emit_chunk(yh3, rloc, W, R, ll, dsts, sigma, out_slice=None, ot=None):
            """One butterfly chunk: R input rows x W per partition.
            ll: [128, R, W] AP holding sigma*ll_true; yh3: a load_yh tile,
            rows rloc:rloc+R used. dsts: (dE,dF,dG,dH) strided [128, R, W]
            target APs."""
            probe = TUNE["probe"]
            if probe == "dma":
                if out_slice is not None:
                    nc.vector.memset(ot[0:1, 0:1, 0:1], 0.0)
                    nc.scalar.dma_start(out=out_slice, in_=ot)
                return
            lh = yh3[:, 0, rloc : rloc + R, :]
            hl = yh3[:, 1, rloc : rloc + R, :]
            hh = yh3[:, 2, rloc : rloc + R, :]

            Cc = abcd.tile([128, R * W], fio, name="Cc", tag="Cc")
            D = abcd.tile([128, R * W], fio, name="D", tag="D")
            C3 = Cc.rearrange("p (r w) -> p r w", w=W)
            D3 = D.rearrange("p (r w) -> p r w", w=W)
            eng = lambda ch: nc.gpsimd if ch == "g" else nc.vector
            # (PE path: hl/hh are raw; sigma enters in the stage-2 stt
            # scalars. DVE path: the whole yh tile was prescaled.)
            cd_e = TUNE["cd_engine"]
            eng(cd_e[0]).tensor_tensor(C3, hl, hh, add)
            eng(cd_e[1]).tensor_tensor(D3, hl, hh, sub)

            dE, dF, dG, dH = dsts
            if use_pe:
                # Height pass on TensorE: A = I@ll + (sigma*I)@lh into PSUM,
                # B likewise with -sigma*I. 512-col matmuls (one PSUM bank),
                # weight swaps minimized.
                Aps = psum.tile([128, R * W], f32, name="Aps", tag="Aps")
                Bps = psum.tile([128, R * W], f32, name="Bps", tag="Bps")
                ll2 = ll.rearrange("p r w -> p (r w)")
                lh2 = lh.rearrange("p r w -> p (r w)")
                wid = w_sb["id"]
                wp = w_sb[(float(sigma), 1.0)]
                wn = w_sb[(float(sigma), -1.0)]
                H2 = R * W // 2
                for ps, dat, wt, st in (
                    (Aps, ll2, wid, True),
                    (Bps, ll2, wid, True),
                    (Aps, lh2, wp, False),
                    (Bps, lh2, wn, False),
                ):
                    for h in range(2):
                        nc.tensor.matmul(
                            ps[:, h * H2 : (h + 1) * H2],
                            wt[:, :],
                            dat[:, h * H2 : (h + 1) * H2],
                            start=st,
                            stop=not st,
                        )
                A3 = Aps.rearrange("p (r w) -> p r w", w=W)
                B3 = Bps.rearrange("p (r w) -> p r w", w=W)
                # Width pass: fused scale+add stt, one PSUM operand each;
                # strided writes do the 2x2 interleave.
                s = float(sigma)
                nc.vector.scalar_tensor_tensor(dE, C3, s, A3, mult, add)
                nc.vector.scalar_tensor_tensor(dF, C3, -s, A3, mult, add)
                nc.vector.scalar_tensor_tensor(dG, D3, s, B3, mult, add)
                nc.vector.scalar_tensor_tensor(dH, D3, -s, B3, mult, add)
            else:
                A = abcd.tile([128, R * W], fio, name="A", tag="A")
                B = abcd.tile([128, R * W], fio, name="B", tag="B")
                A3 = A.rearrange("p (r w) -> p r w", w=W)
                B3 = B.rearrange("p (r w) -> p r w", w=W)
                # Height pass: A = sigma*(ll+lh) (even out rows), B = odd.
                # Dense fp16 TT runs at 2x on DVE; the strided-write width
                # pass is 1x everywhere, so park it on GpSimd by default.
                ab_e = TUNE["ab_engine"]
                eng(ab_e[0]).tensor_tensor(A3, ll, lh, add)
                eng(ab_e[1]).tensor_tensor(B3, ll, lh, sub)
                # Width pass: strided writes do the 2x2 interleave; output
                # carries 2*sigma relative to true.
                se = TUNE["strided_engines"]
                eng(se[0]).tensor_tensor(dE, A3, C3, add)
                eng(se[1]).tensor_tensor(dF, A3, C3, sub)
                eng(se[2]).tensor_tensor(dG, B3, D3, add)
                eng(se[3]).tensor_tensor(dH, B3, D3, sub)

            if out_slice is not None and probe != "compute":
                nc.scalar.dma_start(out=out_slice, in_=ot)

        def interleave_dsts(dst_tile, W, R, r0):
            v = dst_tile.rearrange("p (r ar w ac) -> p ar ac r w", ar=2, ac=2, w=W)
            return (
                v[:, 0, 0, r0 : r0 + R, :],
                v[:, 0, 1, r0 : r0 + R, :],
                v[:, 1, 0, r0 : r0 + R, :],
                v[:, 1, 1, r0 : r0 + R, :],
            )

        for _ in range(reps):  # reps>1 only for benchmarking (device-side loop)
            # Load yl and pre-scale by u0^6 (= 0.125 for Haar).
            yl_tmp = yh_pool.tile(
                [128, 16 * 64], fio, name="yl_tmp", tag="yl_tmp", bufs=1
            )
            yl_v = yl_t[:, :, :].rearrange("c (b r) w -> (c b) r w", b=4)
            if TUNE["probe"] != "compute":
                nc.sync.dma_start(
                    out=yl_tmp.rearrange("p (r w) -> p r w", w=64), in_=yl_v
                )
            else:
                nc.vector.memset(yl_tmp[0:1, 0:1], 0.0)
            if TUNE["probe"] != "dma":
                nc.scalar.mul(llA[:, :], yl_tmp[:, :], float(u0**6))

            # Level A (64x64 -> llB), one chunk.
            llA_v = llA.rearrange("p (r w) -> p r w", w=64)
            yhA = load_yh(yh2_v, 64, 16, 0, float(u0**6))
            emit_chunk(
                yhA, 0, 64, 16,
                llA_v[:, 0:16, :], interleave_dsts(llB, 64, 16, 0),
                float(u0**6),
            )

            # Levels B and C interleaved: each B chunk (8 in-rows -> 16 llC
            # rows per partition) feeds 4 C chunks immediately. With the PE
            # height pass there is no ACT prescale in the load chain, so
            # loads can optionally be batched 2 compute-chunks per DMA
            # (~1MB/channel at level C) without coarsening the pipeline;
            # consumers slice sub-chunks via rloc.
            llB_v = llB.rearrange("p (r w) -> p r w", w=128)
            RB, RC = 8, 4
            batch = 2 if TUNE["batch_loads"] else 1
            yhB = None
            for j in range(4):
                if j % batch == 0:
                    yhB = load_yh(yh1_v, 128, batch * RB, j * RB, float(u0**4))
                seg = segp.tile([128, 16 * 256], fio, name="seg", tag="seg")
                emit_chunk(
                    yhB, (j % batch) * RB, 128, RB,
                    llB_v[:, j * RB : (j + 1) * RB, :],
                    interleave_dsts(seg, 128, RB, 0),
                    float(u0**4),
                )
                seg_v = seg.rearrange("p (r w) -> p r w", w=256)
                yhC = None
                ot2 = None
                for i in range(4):
                    g0r = j * 16 + i * RC  # global C-level input row
                    if i % batch == 0:
                        yhC = load_yh(yh0_v, 256, batch * RC, g0r, float(u0**2))
                    if TUNE["batch_stores"]:
                        # one 4MB store per pair of chunks; the staging tile
                        # lives exactly 2 chunks (no slot-ring hogging).
                        if i % 2 == 0:
                            ot2 = outp.tile(
                                [128, 4 * RC * 512], fio, name="ot", tag="ot"
                            )
                        ot2_r = ot2.rearrange(
                            "p (h r ar w ac) -> p h ar ac r w",
                            h=2, ar=2, ac=2, w=256,
                        )
                        dsts = (
                            ot2_r[:, i % 2, 0, 0],
                            ot2_r[:, i % 2, 0, 1],
                            ot2_r[:, i % 2, 1, 0],
                            ot2_r[:, i % 2, 1, 1],
                        )
                        last = i % 2 == 1
                        emit_chunk(
                            yhC, (i % batch) * RC, 256, RC,
                            seg_v[:, i * RC : (i + 1) * RC, :],
                            dsts,
                            float(u0**2),
                            out_slice=(
                                out_v[:, 2 * (g0r - RC) : 2 * (g0r - RC) + 4 * RC, :]
                                if last
                                else None
                            ),
                            ot=(
                                ot2.rearrange("p (r w) -> p r w", w=512)
                                if last
                                else None
                            ),
                        )
                    else:
                        ot = outp.tile([128, 2 * RC * 512], fio, name="ot", tag="ot")
                        emit_chunk(
                            yhC, (i % batch) * RC, 256, RC,
                            seg_v[:, i * RC : (i + 1) * RC, :],
                            interleave_dsts(ot, 256, RC, 0),
                            float(u0**2),
                            out_slice=out_v[:, 2 * g0r : 2 * g0r + 2 * RC, :],
                            ot=ot.rearrange("p (r w) -> p r w", w=512),
                        )

    nc.compile()
    return nc


def _get_nc(u0, u1, v0, v1):
    key = (round(u0, 9), round(u1, 9), round(v0, 9), round(v1, 9))
    if key not in _cache:
        _cache[key] = _build_program(u0, u1, v0, v1)
    return _cache[key]


def _run(inputs, trace=False, trace_kwargs=None):
    from concourse.bass_utils import run_bass_kernel_spmd

    np_io = np.dtype(TUNE["io_dtype"])
    yl = np.ascontiguousarray(np.asarray(inputs["yl"]).astype(np_io))
    yh0 = np.ascontiguousarray(np.asarray(inputs["yh0"]).astype(np_io))
    yh1 = np.ascontiguousarray(np.asarray(inputs["yh1"]).astype(np_io))
    yh2 = np.ascontiguousarray(np.asarray(inputs["yh2"]).astype(np_io))
    g0 = np.asarray(inputs["g0"], dtype=np.float32)
    g1 = np.asarray(inputs["g1"], dtype=np.float32)

    u0, u1 = float(g0[0]), float(g0[1])
    v0, v1 = float(g1[0]), float(g1[1])

    nc = _get_nc(u0, u1, v0, v1)

    in_maps = [
        {"yl": yl[k], "yh0": yh0[k], "yh1": yh1[k], "yh2": yh2[k]}
        for k in range(N_CORES)
    ]
    kw = {}
    if trace:
        kw["trace"] = True
        if trace_kwargs:
            kw.update(trace_kwargs)
    res = run_bass_kernel_spmd(nc, in_maps, list(range(N_CORES)), **kw)
    out = np.stack([res.results[k]["out"] for k in range(N_CORES)], axis=0)
    return out.astype(np.float32, copy=False), res


def kernel(yl, yh0, yh1, yh2, g0, g1):
    out, _ = _run(
        {"yl": yl, "yh0": yh0, "yh1": yh1, "yh2": yh2, "g0": g0, "g1": g1}
    )
    return out



# revision 14
# speedup vs baseline: 1.0423x; 1.0423x over previous
# Trainium2 Bass kernel: 3-level inverse 2D Haar DWT (DWTInverse, db1, mode=zero).
#
# Math: for a 2-tap synthesis pair (g0=[u0,u1], g1=[v0,v1]) the transposed convs
# have stride 2 and no overlap, so each level is an independent 2x2 butterfly:
#   out[2i+a, 2j+b] = ga[?]... concretely with Haar (u0=u1=v0=a, v1=-a):
#   out[2i,2j]     = 0.5*(ll+lh+hl+hh)
#   out[2i,2j+1]   = 0.5*(ll+lh-hl-hh)
#   out[2i+1,2j]   = 0.5*(ll-lh+hl-hh)
#   out[2i+1,2j+1] = 0.5*(ll-lh-hl+hh)
# Shapes (64->128->256->512 with matching yh sizes) never trigger the crop branch.
#
# Sharding: pure data parallel over batch N=8 -> core k processes n=k
# (32 channels x full spatial). No cross-core communication.
#
# Layout per core: SBUF partition p = (c, b) = c*4+b, c in [0,32) channels,
# b in [0,4) row-blocks of each image. Row-blocks are butterfly-invariant
# (input rows of block b map to output rows of block b), so every level is
# purely free-dim work with strided writes doing the 2x2 spatial interleave
# for free. Intermediates stay in SBUF (llB resident; level-B output in a
# 2-deep sliding segment ring).
#
# fp16 everywhere at the HBM edge (rel-err gate is 2e-2; fp16 end-to-end
# measures ~7e-4): kernel() casts inputs to fp16 host-side, the device
# writes an fp16 output, host upcasts. Halves HBM traffic to 16MB in +
# 16MB out per core. Measured per-rep steady state (paired device-resident
# slope, R=65): full ~76us, DMA-only ~73us (= 32MB / 438GB/s, exactly the
# 16-port SBUF AXI fabric limit shared by both HWDGE rings), compute-only
# ~77us (DVE-bound: 84 stage-2 stt ops). Both rooflines met and fully
# overlapped; fp32 baseline measured ~185us with the same method.
#
# Scale folding: intermediates carry sigma*true (llA 1/8, llB 1/4, seg 1/2).
# Stage-1 (height pass) runs on TensorE as diagonal-weight matmuls into
# PSUM (sigma folded into the fp16 weights; exact powers of 1/2 for Haar);
# C/D (hi-branch height pass) on GpSimd; stage-2 (width pass) is 4 fused
# scale+add stt ops on DVE with the PSUM operand and strided fp16 writes.
# GpSimd cannot read PSUM, so stage-2 is DVE-only by construction; measured
# alternatives (vector-only butterfly with GpSimd interleave writes, ACT
# prescale, cd_engine=vv, batch loads/stores) were all slower on HW.

import numpy as np
from contextlib import ExitStack

C_PER_CORE = 32
N_CORES = 8

_cache = {}

# Tuning knobs (read at build time; bench scripts override before building).
TUNE = {
    "yh_bufs": 5,
    "out_bufs": 4,
    "seg_bufs": 2,
    "cd_bufs": 3,
    "stage1": "pe",  # "dve" (vector butterfly) | "pe" (height pass on TensorE)
    "batch_loads": False,  # 2 compute-chunks per load DMA (~1MB/channel)
    "batch_stores": False,  # 2 compute-chunks per store DMA (4MB)
    "probe": None,  # None | "dma" (transfers only) | "compute" (engines only)
    "io_dtype": "float16",  # HBM/SBUF dtype; "float32" | "float16" (host casts)
    # Engine assignment (dve path): "v"=DVE, "g"=GpSimd.
    "prescale_engine": "v",  # yh tile sigma prescale: "v" (TS 4x) | "a" (ACT)
    "ab_engine": "vv",  # stage-1 A,B (dense TT, 2x on DVE for fp16)
    "cd_engine": "gg",  # stage-1 C,D (dense TT; GpSimd frees DVE for stage-2)
    "strided_engines": "gggg",  # E,F,G,H interleave writes (1x everywhere)
}


def _build_program(u0, u1, v0, v1, reps=1):
    import concourse.bacc as bacc
    import concourse.mybir as mybir
    import concourse.tile as tile

    f32 = mybir.dt.float32
    fio = getattr(mybir.dt, TUNE["io_dtype"])
    np_fio = mybir.dt.np(fio)
    mult = mybir.AluOpType.mult
    add = mybir.AluOpType.add

    rA = v0 / u0  # +1 for Haar (even taps)
    rB = v1 / u1  # -1 for Haar (odd taps)

    # Bacc (not bass.Bass): its compile() runs generate_event_semaphores,
    # which splits multi-sem waits — TPB instructions can carry only ONE
    # sync wait, and walrus codegen hard-errors otherwise.
    nc = bacc.Bacc(
        "TRN2",
        target_bir_lowering=False,
        debug=False,
        enable_asserts=False,
        num_devices=N_CORES,
    )
    yl_t = nc.dram_tensor("yl", [C_PER_CORE, 64, 64], fio, kind="ExternalInput")
    yh0_t = nc.dram_tensor("yh0", [C_PER_CORE, 3, 256, 256], fio, kind="ExternalInput")
    yh1_t = nc.dram_tensor("yh1", [C_PER_CORE, 3, 128, 128], fio, kind="ExternalInput")
    yh2_t = nc.dram_tensor("yh2", [C_PER_CORE, 3, 64, 64], fio, kind="ExternalInput")
    out_t = nc.dram_tensor("out", [C_PER_CORE, 512, 512], fio, kind="ExternalOutput")

    assert abs(rA - 1.0) < 1e-6 and abs(rB + 1.0) < 1e-6
    sub = mybir.AluOpType.subtract
    use_pe = TUNE["stage1"] == "pe"
    sigmas = [float(u0**6), float(u0**4), float(u0**2)]
    if use_pe:
        # Diagonal weight matrices for the PE height pass: identity plus
        # +/- sigma*I per level, embedded in the NEFF as Const tensors.
        # (sigma = powers of 1/2 for Haar -> exact in fp16.)
        w_dram = {"id": nc.inline_tensor(np.eye(128, dtype=np_fio), "w_id")}
        for si, s in enumerate(sigmas):
            for sgn in (1.0, -1.0):
                w_dram[(s, sgn)] = nc.inline_tensor(
                    (sgn * s * np.eye(128)).astype(np_fio),
                    f"w_{si}_{'p' if sgn > 0 else 'n'}",
                )

    with ExitStack() as ctx:
        tc = ctx.enter_context(tile.TileContext(nc))
        res = ctx.enter_context(tc.tile_pool(name="res", bufs=1))
        if use_pe:
            psum = ctx.enter_context(
                tc.tile_pool(name="psum", bufs=2, space="PSUM")
            )
            w_sb = {}
            for key, dh in w_dram.items():
                wt = res.tile(
                    [128, 128], fio, name="wt", tag=f"w{len(w_sb)}"
                )
                nc.sync.dma_start(out=wt[:, :], in_=dh[:, :])
                w_sb[key] = wt
        yh_pool = ctx.enter_context(tc.tile_pool(name="yh", bufs=TUNE["yh_bufs"]))
        abcd = ctx.enter_context(tc.tile_pool(name="abcd", bufs=TUNE["cd_bufs"]))
        outp = ctx.enter_context(tc.tile_pool(name="outp", bufs=TUNE["out_bufs"]))
        segp = ctx.enter_context(tc.tile_pool(name="segp", bufs=TUNE["seg_bufs"]))

        # Resident ll tiles (per partition: rows of my block, dense row-major).
        llA = res.tile([128, 16 * 64], fio, name="llA")     # 0.125 * yl
        llB = res.tile([128, 32 * 128], fio, name="llB")    # 0.25 * level-A out
        # level-B output (0.5 * ll_C) lives in a sliding ring of 2 segments
        # of 16 rows x 256 (llC_seg) so level C can start while level B is
        # still producing, and the freed SBUF buys deeper yh prefetch.

        yh0_v = yh0_t[:, :, :, :].rearrange("c k (b r) w -> c k b r w", b=4)
        yh1_v = yh1_t[:, :, :, :].rearrange("c k (b r) w -> c k b r w", b=4)
        yh2_v = yh2_t[:, :, :, :].rearrange("c k (b r) w -> c k b r w", b=4)
        out_v = out_t[:, :, :].rearrange("c (b r) w -> (c b) r w", b=4)

        def load_yh(yh_v, W, RL, r0, sigma):
            """Load RL rows x W of all 3 detail channels in one 3-dim DMA
            per channel and pre-scale the whole tile by sigma on ACT (one
            ACT op; it also funnels the 3 DMA semaphores into one ACT
            semaphore for all downstream consumers). Returns [p,k,r,w]."""
            probe = TUNE["probe"]
            yh_tile = yh_pool.tile([128, 3 * RL * W], fio, name="yh_tile", tag="yh")
            yh3 = yh_tile.rearrange("p (k r w) -> p k r w", k=3, r=RL)
            if probe != "compute":
                for k in range(3):
                    nc.sync.dma_start(
                        out=yh3[:, k], in_=yh_v[:, k, :, r0 : r0 + RL, :]
                    )
            else:
                # probe tiles need a writer to get a slot allocated
                nc.vector.memset(yh_tile[0:1, 0:1], 0.0)
            if probe != "dma" and not use_pe:
                # DVE stage1 wants pre-scaled details; the PE path folds
                # sigma into the matmul weights / stt scalars instead.
                # TS on DVE hits 4x perf mode for dense fp16 (0.26 ns/elem).
                if TUNE["prescale_engine"] == "v":
                    nc.vector.tensor_scalar(
                        yh_tile[:, :], yh_tile[:, :], float(sigma), None, mult
                    )
                else:
                    nc.scalar.mul(yh_tile[:, :], yh_tile[:, :], float(sigma))
            return yh3

        def emit_chunk(yh3, rloc, W, R, ll, dsts, sigma, out_slice=None, ot=None):
            """One butterfly chunk: R input rows x W per partition.
            ll: [128, R, W] AP holding sigma*ll_true; yh3: a load_yh tile,
            rows rloc:rloc+R used. dsts: (dE,dF,dG,dH) strided [128, R, W]
            target APs."""
            probe = TUNE["probe"]
            if probe == "dma":
                if out_slice is not None:
                    nc.vector.memset(ot[0:1, 0:1, 0:1], 0.0)
                    nc.scalar.dma_start(out=out_slice, in_=ot)
                return
            lh = yh3[:, 0, rloc : rloc + R, :]
            hl = yh3[:, 1, rloc : rloc + R, :]
            hh = yh3[:, 2, rloc : rloc + R, :]

            Cc = abcd.tile([128, R * W], fio, name="Cc", tag="Cc")
            D = abcd.tile([128, R * W], fio, name="D", tag="D")
            C3 = Cc.rearrange("p (r w) -> p r w", w=W)
            D3 = D.rearrange("p (r w) -> p r w", w=W)
            eng = lambda ch: nc.gpsimd if ch == "g" else nc.vector
            # (PE path: hl/hh are raw; sigma enters in the stage-2 stt
            # scalars. DVE path: the whole yh tile was prescaled.)
            cd_e = TUNE["cd_engine"]
            eng(cd_e[0]).tensor_tensor(C3, hl, hh, add)
            eng(cd_e[1]).tensor_tensor(D3, hl, hh, sub)

            dE, dF, dG, dH = dsts
            if use_pe:
                # Height pass on TensorE: A = I@ll + (sigma*I)@lh into PSUM,
                # B likewise with -sigma*I. 512-col matmuls (one PSUM bank),
                # weight swaps minimized.
                Aps = psum.tile([128, R * W], f32, name="Aps", tag="Aps")
                Bps = psum.tile([128, R * W], f32, name="Bps", tag="Bps")
                ll2 = ll.rearrange("p r w -> p (r w)")
                lh2 = lh.rearrange("p r w -> p (r w)")
                wid = w_sb["id"]
                wp = w_sb[(float(sigma), 1.0)]
                wn = w_sb[(float(sigma), -1.0)]
                H2 = R * W // 2
                for ps, dat, wt, st in (
                    (Aps, ll2, wid, True),
                    (Bps, ll2, wid, True),
                    (Aps, lh2, wp, False),
                    (Bps, lh2, wn, False),
                ):
                    for h in range(2):
                        nc.tensor.matmul(
                            ps[:, h * H2 : (h + 1) * H2],
                            wt[:, :],
                            dat[:, h * H2 : (h + 1) * H2],
                            start=st,
                            stop=not st,
                        )
                A3 = Aps.rearrange("p (r w) -> p r w", w=W)
                B3 = Bps.rearrange("p (r w) -> p r w", w=W)
                # Width pass: fused scale+add stt, one PSUM operand each;
                # strided writes do the 2x2 interleave.
                s = float(sigma)
                nc.vector.scalar_tensor_tensor(dE, C3, s, A3, mult, add)
                nc.vector.scalar_tensor_tensor(dF, C3, -s, A3, mult, add)
                nc.vector.scalar_tensor_tensor(dG, D3, s, B3, mult, add)
                nc.vector.scalar_tensor_tensor(dH, D3, -s, B3, mult, add)
            else:
                A = abcd.tile([128, R * W], fio, name="A", tag="A")
                B = abcd.tile([128, R * W], fio, name="B", tag="B")
                A3 = A.rearrange("p (r w) -> p r w", w=W)
                B3 = B.rearrange("p (r w) -> p r w", w=W)
                # Height pass: A = sigma*(ll+lh) (even out rows), B = odd.
                # Dense fp16 TT runs at 2x on DVE; the strided-write width
                # pass is 1x everywhere, so park it on GpSimd by default.
                ab_e = TUNE["ab_engine"]
                eng(ab_e[0]).tensor_tensor(A3, ll, lh, add)
                eng(ab_e[1]).tensor_tensor(B3, ll, lh, sub)
                # Width pass: strided writes do the 2x2 interleave; output
                # carries 2*sigma relative to true.
                se = TUNE["strided_engines"]
                eng(se[0]).tensor_tensor(dE, A3, C3, add)
                eng(se[1]).tensor_tensor(dF, A3, C3, sub)
                eng(se[2]).tensor_tensor(dG, B3, D3, add)
                eng(se[3]).tensor_tensor(dH, B3, D3, sub)

            if out_slice is not None and probe != "compute":
                nc.scalar.dma_start(out=out_slice, in_=ot)

        def interleave_dsts(dst_tile, W, R, r0):
            v = dst_tile.rearrange("p (r ar w ac) -> p ar ac r w", ar=2, ac=2, w=W)
            return (
                v[:, 0, 0, r0 : r0 + R, :],
                v[:, 0, 1, r0 : r0 + R, :],
                v[:, 1, 0, r0 : r0 + R, :],
                v[:, 1, 1, r0 : r0 + R, :],
            )

        for _ in range(reps):  # reps>1 only for benchmarking (device-side loop)
            # Load yl and pre-scale by u0^6 (= 0.125 for Haar).
            yl_tmp = yh_pool.tile(
                [128, 16 * 64], fio, name="yl_tmp", tag="yl_tmp", bufs=1
            )
            yl_v = yl_t[:, :, :].rearrange("c (b r) w -> (c b) r w", b=4)
            if TUNE["probe"] != "compute":
                nc.sync.dma_start(
                    out=yl_tmp.rearrange("p (r w) -> p r w", w=64), in_=yl_v
                )
            else:
                nc.vector.memset(yl_tmp[0:1, 0:1], 0.0)
            if TUNE["probe"] != "dma":
                nc.scalar.mul(llA[:, :], yl_tmp[:, :], float(u0**6))

            # Level A (64x64 -> llB), one chunk.
            llA_v = llA.rearrange("p (r w) -> p r w", w=64)
            yhA = load_yh(yh2_v, 64, 16, 0, float(u0**6))
            emit_chunk(
                yhA, 0, 64, 16,
                llA_v[:, 0:16, :], interleave_dsts(llB, 64, 16, 0),
                float(u0**6),
            )

            # Levels B and C interleaved: each B chunk (8 in-rows -> 16 llC
            # rows per partition) feeds 4 C chunks immediately. With the PE
            # height pass there is no ACT prescale in the load chain, so
            # loads can optionally be batched 2 compute-chunks per DMA
            # (~1MB/channel at level C) without coarsening the pipeline;
            # consumers slice sub-chunks via rloc.
            llB_v = llB.rearrange("p (r w) -> p r w", w=128)
            RB, RC = 8, 4
            batch = 2 if TUNE["batch_loads"] else 1
            yhB = None
            for j in range(4):
                if j % batch == 0:
                    yhB = load_yh(yh1_v, 128, batch * RB, j * RB, float(u0**4))
                seg = segp.tile([128, 16 * 256], fio, name="seg", tag="seg")
                emit_chunk(
                    yhB, (j % batch) * RB, 128, RB,
                    llB_v[:, j * RB : (j + 1) * RB, :],
                    interleave_dsts(seg, 128, RB, 0),
                    float(u0**4),
                )
                seg_v = seg.rearrange("p (r w) -> p r w", w=256)
                yhC = None
                ot2 = None
                for i in range(4):
                    g0r = j * 16 + i * RC  # global C-level input row
                    if i % batch == 0:
                        yhC = load_yh(yh0_v, 256, batch * RC, g0r, float(u0**2))
                    if TUNE["batch_stores"]:
                        # one 4MB store per pair of chunks; the staging tile
                        # lives exactly 2 chunks (no slot-ring hogging).
                        if i % 2 == 0:
                            ot2 = outp.tile(
                                [128, 4 * RC * 512], fio, name="ot", tag="ot"
                            )
                        ot2_r = ot2.rearrange(
                            "p (h r ar w ac) -> p h ar ac r w",
                            h=2, ar=2, ac=2, w=256,
                        )
                        dsts = (
                            ot2_r[:, i % 2, 0, 0],
                            ot2_r[:, i % 2, 0, 1],
                            ot2_r[:, i % 2, 1, 0],
                            ot2_r[:, i % 2, 1, 1],
                        )
                        last = i % 2 == 1
                        emit_chunk(
                            yhC, (i % batch) * RC, 256, RC,
                            seg_v[:, i * RC : (i + 1) * RC, :],
                            dsts,
                            float(u0**2),
                            out_slice=(
                                out_v[:, 2 * (g0r - RC) : 2 * (g0r - RC) + 4 * RC, :]
                                if last
                                else None
                            ),
                            ot=(
                                ot2.rearrange("p (r w) -> p r w", w=512)
                                if last
                                else None
                            ),
                        )
                    else:
                        ot = outp.tile([128, 2 * RC * 512], fio, name="ot", tag="ot")
                        emit_chunk(
                            yhC, (i % batch) * RC, 256, RC,
                            seg_v[:, i * RC : (i + 1) * RC, :],
                            interleave_dsts(ot, 256, RC, 0),
                            float(u0**2),
                            out_slice=out_v[:, 2 * g0r : 2 * g0r + 2 * RC, :],
                            ot=ot.rearrange("p (r w) -> p r w", w=512),
                        )

    nc.compile()
    return nc


def _get_nc(u0, u1, v0, v1):
    key = (round(u0, 9), round(u1, 9), round(v0, 9), round(v1, 9))
    if key not in _cache:
        _cache[key] = _build_program(u0, u1, v0, v1)
    return _cache[key]


def _np_io():
    """numpy dtype for TUNE["io_dtype"] (bfloat16 needs ml_dtypes)."""
    if TUNE["io_dtype"] == "bfloat16":
        import ml_dtypes

        return np.dtype(ml_dtypes.bfloat16)
    return np.dtype(TUNE["io_dtype"])


def _run(inputs, trace=False, trace_kwargs=None):
    from concourse.bass_utils import run_bass_kernel_spmd

    np_io = _np_io()
    yl = np.ascontiguousarray(np.asarray(inputs["yl"]).astype(np_io))
    yh0 = np.ascontiguousarray(np.asarray(inputs["yh0"]).astype(np_io))
    yh1 = np.ascontiguousarray(np.asarray(inputs["yh1"]).astype(np_io))
    yh2 = np.ascontiguousarray(np.asarray(inputs["yh2"]).astype(np_io))
    g0 = np.asarray(inputs["g0"], dtype=np.float32)
    g1 = np.asarray(inputs["g1"], dtype=np.float32)

    u0, u1 = float(g0[0]), float(g0[1])
    v0, v1 = float(g1[0]), float(g1[1])

    nc = _get_nc(u0, u1, v0, v1)

    in_maps = [
        {"yl": yl[k], "yh0": yh0[k], "yh1": yh1[k], "yh2": yh2[k]}
        for k in range(N_CORES)
    ]
    kw = {}
    if trace:
        kw["trace"] = True
        if trace_kwargs:
            kw.update(trace_kwargs)
    res = run_bass_kernel_spmd(nc, in_maps, list(range(N_CORES)), **kw)
    out = np.stack([res.results[k]["out"] for k in range(N_CORES)], axis=0)
    return out.astype(np.float32, copy=False), res


def kernel(yl, yh0, yh1, yh2, g0, g1):
    out, _ = _run(
        {"yl": yl, "yh0": yh0, "yh1": yh1, "yh2": yh2, "g0": g0, "g1": g1}
    )
    return out



# revision 20
# speedup vs baseline: 1.0813x; 1.0374x over previous
# Trainium2 Bass kernel: 3-level inverse 2D Haar DWT (DWTInverse, db1, mode=zero).
#
# Math: for a 2-tap synthesis pair (g0=[u0,u1], g1=[v0,v1]) the transposed convs
# have stride 2 and no overlap, so each level is an independent 2x2 butterfly:
#   out[2i+a, 2j+b] = ga[?]... concretely with Haar (u0=u1=v0=a, v1=-a):
#   out[2i,2j]     = 0.5*(ll+lh+hl+hh)
#   out[2i,2j+1]   = 0.5*(ll+lh-hl-hh)
#   out[2i+1,2j]   = 0.5*(ll-lh+hl-hh)
#   out[2i+1,2j+1] = 0.5*(ll-lh-hl+hh)
# Shapes (64->128->256->512 with matching yh sizes) never trigger the crop branch.
#
# Sharding: pure data parallel over batch N=8 -> core k processes n=k
# (32 channels x full spatial). No cross-core communication.
#
# Layout per core: SBUF partition p = (c, b) = c*4+b, c in [0,32) channels,
# b in [0,4) row-blocks of each image. Row-blocks are butterfly-invariant
# (input rows of block b map to output rows of block b), so every level is
# purely free-dim work with strided writes doing the 2x2 spatial interleave
# for free. Intermediates stay in SBUF (llB resident; level-B output in a
# 2-deep sliding segment ring).
#
# fp16 everywhere at the HBM edge (rel-err gate is 2e-2; fp16 end-to-end
# measures ~7e-4): kernel() casts inputs to fp16 host-side, the device
# writes an fp16 output, host upcasts. Halves HBM traffic to 16MB in +
# 16MB out per core. Measured per-rep steady state (paired device-resident
# slope, R=65): full ~76us, DMA-only ~73us (= 32MB / 438GB/s, exactly the
# 16-port SBUF AXI fabric limit shared by both HWDGE rings), compute-only
# ~77us (DVE-bound: 84 stage-2 stt ops). Both rooflines met and fully
# overlapped; fp32 baseline measured ~185us with the same method.
#
# Scale folding: intermediates carry sigma*true (llA 1/8, llB 1/4, seg 1/2).
# Stage-1 (height pass) runs on TensorE as diagonal-weight matmuls into
# PSUM (sigma folded into the fp16 weights; exact powers of 1/2 for Haar);
# C/D (hi-branch height pass) on GpSimd; stage-2 (width pass) is 4 fused
# scale+add stt ops on DVE with the PSUM operand and strided fp16 writes.
# GpSimd cannot read PSUM, so stage-2 is DVE-only by construction; measured
# alternatives (vector-only butterfly with GpSimd interleave writes, ACT
# prescale, cd_engine=vv, batch loads/stores) were all slower on HW.

import numpy as np
from contextlib import ExitStack

C_PER_CORE = 32
N_CORES = 8

_cache = {}

# Tuning knobs (read at build time; bench scripts override before building).
TUNE = {
    "yh_bufs": 5,
    "out_bufs": 4,
    "seg_bufs": 2,
    "cd_bufs": 3,
    "stage1": "pe",  # "dve" (vector butterfly) | "pe" (height pass on TensorE)
    "batch_loads": False,  # 2 compute-chunks per load DMA (~1MB/channel)
    "batch_stores": False,  # 2 compute-chunks per store DMA (4MB)
    "probe": None,  # None | "dma" (transfers only) | "compute" (engines only)
    "io_dtype": "float16",  # HBM/SBUF dtype; "float32" | "float16" (host casts)
    # Engine assignment (dve path): "v"=DVE, "g"=GpSimd.
    "prescale_engine": "v",  # yh tile sigma prescale: "v" (TS 4x) | "a" (ACT)
    # fuse_stt=True halves DVE op count via row-interleaved 3D-AP stt pairs;
    # CoreSim-correct but neuronxcc rejects it (stt inputs must be <=3D:
    # NCC_IBIR133), and a 3D-legal layout would need matmul outputs spanning
    # PSUM banks. Keep False on hardware.
    "fuse_stt": False,
    "ab_engine": "vv",  # stage-1 A,B (dense TT, 2x on DVE for fp16)
    "cd_engine": "gg",  # stage-1 C,D (dense TT; GpSimd frees DVE for stage-2)
    "strided_engines": "gggg",  # E,F,G,H interleave writes (1x everywhere)
}


def _build_program(u0, u1, v0, v1, reps=1):
    import concourse.bacc as bacc
    import concourse.mybir as mybir
    import concourse.tile as tile

    f32 = mybir.dt.float32
    fio = getattr(mybir.dt, TUNE["io_dtype"])
    np_fio = mybir.dt.np(fio)
    mult = mybir.AluOpType.mult
    add = mybir.AluOpType.add

    rA = v0 / u0  # +1 for Haar (even taps)
    rB = v1 / u1  # -1 for Haar (odd taps)

    # Bacc (not bass.Bass): its compile() runs generate_event_semaphores,
    # which splits multi-sem waits — TPB instructions can carry only ONE
    # sync wait, and walrus codegen hard-errors otherwise.
    nc = bacc.Bacc(
        "TRN2",
        target_bir_lowering=False,
        debug=False,
        enable_asserts=False,
        num_devices=N_CORES,
    )
    yl_t = nc.dram_tensor("yl", [C_PER_CORE, 64, 64], fio, kind="ExternalInput")
    yh0_t = nc.dram_tensor("yh0", [C_PER_CORE, 3, 256, 256], fio, kind="ExternalInput")
    yh1_t = nc.dram_tensor("yh1", [C_PER_CORE, 3, 128, 128], fio, kind="ExternalInput")
    yh2_t = nc.dram_tensor("yh2", [C_PER_CORE, 3, 64, 64], fio, kind="ExternalInput")
    out_t = nc.dram_tensor("out", [C_PER_CORE, 512, 512], fio, kind="ExternalOutput")

    assert abs(rA - 1.0) < 1e-6 and abs(rB + 1.0) < 1e-6
    sub = mybir.AluOpType.subtract
    use_pe = TUNE["stage1"] == "pe"
    sigmas = [float(u0**6), float(u0**4), float(u0**2)]
    if use_pe:
        # Diagonal weight matrices for the PE height pass: identity plus
        # +/- sigma*I per level, embedded in the NEFF as Const tensors.
        # (sigma = powers of 1/2 for Haar -> exact in fp16.)
        w_dram = {"id": nc.inline_tensor(np.eye(128, dtype=np_fio), "w_id")}
        for si, s in enumerate(sigmas):
            for sgn in (1.0, -1.0):
                w_dram[(s, sgn)] = nc.inline_tensor(
                    (sgn * s * np.eye(128)).astype(np_fio),
                    f"w_{si}_{'p' if sgn > 0 else 'n'}",
                )

    with ExitStack() as ctx:
        tc = ctx.enter_context(tile.TileContext(nc))
        res = ctx.enter_context(tc.tile_pool(name="res", bufs=1))
        if use_pe:
            psum = ctx.enter_context(
                tc.tile_pool(name="psum", bufs=2, space="PSUM")
            )
            w_sb = {}
            for key, dh in w_dram.items():
                wt = res.tile(
                    [128, 128], fio, name="wt", tag=f"w{len(w_sb)}"
                )
                nc.sync.dma_start(out=wt[:, :], in_=dh[:, :])
                w_sb[key] = wt
        yh_pool = ctx.enter_context(tc.tile_pool(name="yh", bufs=TUNE["yh_bufs"]))
        abcd = ctx.enter_context(tc.tile_pool(name="abcd", bufs=TUNE["cd_bufs"]))
        outp = ctx.enter_context(tc.tile_pool(name="outp", bufs=TUNE["out_bufs"]))
        segp = ctx.enter_context(tc.tile_pool(name="segp", bufs=TUNE["seg_bufs"]))

        # Resident ll tiles (per partition: rows of my block, dense row-major).
        llA = res.tile([128, 16 * 64], fio, name="llA")     # 0.125 * yl
        llB = res.tile([128, 32 * 128], fio, name="llB")    # 0.25 * level-A out
        # level-B output (0.5 * ll_C) lives in a sliding ring of 2 segments
        # of 16 rows x 256 (llC_seg) so level C can start while level B is
        # still producing, and the freed SBUF buys deeper yh prefetch.

        yh0_v = yh0_t[:, :, :, :].rearrange("c k (b r) w -> c k b r w", b=4)
        yh1_v = yh1_t[:, :, :, :].rearrange("c k (b r) w -> c k b r w", b=4)
        yh2_v = yh2_t[:, :, :, :].rearrange("c k (b r) w -> c k b r w", b=4)
        out_v = out_t[:, :, :].rearrange("c (b r) w -> (c b) r w", b=4)

        def load_yh(yh_v, W, RL, r0, sigma):
            """Load RL rows x W of all 3 detail channels in one 3-dim DMA
            per channel and pre-scale the whole tile by sigma on ACT (one
            ACT op; it also funnels the 3 DMA semaphores into one ACT
            semaphore for all downstream consumers). Returns [p,k,r,w]."""
            probe = TUNE["probe"]
            yh_tile = yh_pool.tile([128, 3 * RL * W], fio, name="yh_tile", tag="yh")
            yh3 = yh_tile.rearrange("p (k r w) -> p k r w", k=3, r=RL)
            if probe != "compute":
                for k in range(3):
                    nc.sync.dma_start(
                        out=yh3[:, k], in_=yh_v[:, k, :, r0 : r0 + RL, :]
                    )
            else:
                # probe tiles need a writer to get a slot allocated
                nc.vector.memset(yh_tile[0:1, 0:1], 0.0)
            if probe != "dma" and not use_pe:
                # DVE stage1 wants pre-scaled details; the PE path folds
                # sigma into the matmul weights / stt scalars instead.
                # TS on DVE hits 4x perf mode for dense fp16 (0.26 ns/elem).
                if TUNE["prescale_engine"] == "v":
                    nc.vector.tensor_scalar(
                        yh_tile[:, :], yh_tile[:, :], float(sigma), None, mult
                    )
                else:
                    nc.scalar.mul(yh_tile[:, :], yh_tile[:, :], float(sigma))
            return yh3

        def emit_chunk(yh3, rloc, W, R, ll, dsts, sigma, out_slice=None, ot=None):
            """One butterfly chunk: R input rows x W per partition.
            ll: [128, R, W] AP holding sigma*ll_true; yh3: a load_yh tile,
            rows rloc:rloc+R used. dsts: (dE,dF,dG,dH) strided [128, R, W]
            target APs."""
            probe = TUNE["probe"]
            if probe == "dma":
                if out_slice is not None:
                    nc.vector.memset(ot[0:1, 0:1, 0:1], 0.0)
                    nc.scalar.dma_start(out=out_slice, in_=ot)
                return
            lh = yh3[:, 0, rloc : rloc + R, :]
            hl = yh3[:, 1, rloc : rloc + R, :]
            hh = yh3[:, 2, rloc : rloc + R, :]

            fuse = use_pe and TUNE["fuse_stt"]
            if fuse:
                # C,D in one tile so the fused stt can read them with a
                # single row-interleaved 3D AP.
                CD = abcd.tile([128, 2 * R * W], fio, name="CD", tag="CD")
                C3 = CD[:, 0 : R * W].rearrange("p (r w) -> p r w", w=W)
                D3 = CD[:, R * W : 2 * R * W].rearrange("p (r w) -> p r w", w=W)
            else:
                Cc = abcd.tile([128, R * W], fio, name="Cc", tag="Cc")
                D = abcd.tile([128, R * W], fio, name="D", tag="D")
                C3 = Cc.rearrange("p (r w) -> p r w", w=W)
                D3 = D.rearrange("p (r w) -> p r w", w=W)
            eng = lambda ch: nc.gpsimd if ch == "g" else nc.vector
            # (PE path: hl/hh are raw; sigma enters in the stage-2 stt
            # scalars. DVE path: the whole yh tile was prescaled.)
            cd_e = TUNE["cd_engine"]
            eng(cd_e[0]).tensor_tensor(C3, hl, hh, add)
            eng(cd_e[1]).tensor_tensor(D3, hl, hh, sub)

            dE, dF, dG, dH = dsts[:4]
            if use_pe:
                # Height pass on TensorE: A = I@ll + (sigma*I)@lh into PSUM,
                # B likewise with -sigma*I. 512-col matmuls (one PSUM bank),
                # weight swaps minimized. Fused mode keeps A,B in one PSUM
                # tile so stage-2 is 2 stt ops (E|G share +sigma, F|H share
                # -sigma) over row-interleaved 3D APs — halves DVE per-op
                # overhead (decode/dispatch/PSUM-init/drain).
                RW = R * W
                ll2 = ll.rearrange("p r w -> p (r w)")
                lh2 = lh.rearrange("p r w -> p (r w)")
                wid = w_sb["id"]
                wp = w_sb[(float(sigma), 1.0)]
                wn = w_sb[(float(sigma), -1.0)]
                H2 = RW // 2
                if fuse:
                    ABps = psum.tile([128, 2 * RW], f32, name="ABps", tag="AB")
                    offs = ((0, wid, True), (RW, wid, True), (0, wp, False), (RW, wn, False))
                    dats = (ll2, ll2, lh2, lh2)
                    for (off, wt, st), dat in zip(offs, dats):
                        for h in range(2):
                            nc.tensor.matmul(
                                ABps[:, off + h * H2 : off + (h + 1) * H2],
                                wt[:, :],
                                dat[:, h * H2 : (h + 1) * H2],
                                start=st,
                                stop=not st,
                            )
                    s = float(sigma)
                    AB_i = ABps.rearrange("p (ab r w) -> p r ab w", ab=2, w=W)
                    CD_i = CD.rearrange("p (ab r w) -> p r ab w", ab=2, w=W)
                    nc.vector.scalar_tensor_tensor(dsts[4], CD_i, s, AB_i, mult, add)
                    nc.vector.scalar_tensor_tensor(dsts[5], CD_i, -s, AB_i, mult, add)
                else:
                    Aps = psum.tile([128, RW], f32, name="Aps", tag="Aps")
                    Bps = psum.tile([128, RW], f32, name="Bps", tag="Bps")
                    for ps, dat, wt, st in (
                        (Aps, ll2, wid, True),
                        (Bps, ll2, wid, True),
                        (Aps, lh2, wp, False),
                        (Bps, lh2, wn, False),
                    ):
                        for h in range(2):
                            nc.tensor.matmul(
                                ps[:, h * H2 : (h + 1) * H2],
                                wt[:, :],
                                dat[:, h * H2 : (h + 1) * H2],
                                start=st,
                                stop=not st,
                            )
                    A3 = Aps.rearrange("p (r w) -> p r w", w=W)
                    B3 = Bps.rearrange("p (r w) -> p r w", w=W)
                    # Width pass: fused scale+add stt, one PSUM operand each;
                    # strided writes do the 2x2 interleave.
                    s = float(sigma)
                    nc.vector.scalar_tensor_tensor(dE, C3, s, A3, mult, add)
                    nc.vector.scalar_tensor_tensor(dF, C3, -s, A3, mult, add)
                    nc.vector.scalar_tensor_tensor(dG, D3, s, B3, mult, add)
                    nc.vector.scalar_tensor_tensor(dH, D3, -s, B3, mult, add)
            else:
                A = abcd.tile([128, R * W], fio, name="A", tag="A")
                B = abcd.tile([128, R * W], fio, name="B", tag="B")
                A3 = A.rearrange("p (r w) -> p r w", w=W)
                B3 = B.rearrange("p (r w) -> p r w", w=W)
                # Height pass: A = sigma*(ll+lh) (even out rows), B = odd.
                # Dense fp16 TT runs at 2x on DVE; the strided-write width
                # pass is 1x everywhere, so park it on GpSimd by default.
                ab_e = TUNE["ab_engine"]
                eng(ab_e[0]).tensor_tensor(A3, ll, lh, add)
                eng(ab_e[1]).tensor_tensor(B3, ll, lh, sub)
                # Width pass: strided writes do the 2x2 interleave; output
                # carries 2*sigma relative to true.
                se = TUNE["strided_engines"]
                eng(se[0]).tensor_tensor(dE, A3, C3, add)
                eng(se[1]).tensor_tensor(dF, A3, C3, sub)
                eng(se[2]).tensor_tensor(dG, B3, D3, add)
                eng(se[3]).tensor_tensor(dH, B3, D3, sub)

            if out_slice is not None and probe != "compute":
                nc.scalar.dma_start(out=out_slice, in_=ot)

        def interleave_dsts(dst_tile, W, R, r0):
            v = dst_tile.rearrange("p (r ar w ac) -> p ar ac r w", ar=2, ac=2, w=W)
            # Pair views for the fused stt: all rows, one column parity —
            # traversal (r, ar, w) matches the AB/CD interleaved reads.
            v2 = dst_tile.rearrange("p (r ar w ac) -> p ac r ar w", ar=2, ac=2, w=W)
            return (
                v[:, 0, 0, r0 : r0 + R, :],
                v[:, 0, 1, r0 : r0 + R, :],
                v[:, 1, 0, r0 : r0 + R, :],
                v[:, 1, 1, r0 : r0 + R, :],
                v2[:, 0, r0 : r0 + R],
                v2[:, 1, r0 : r0 + R],
            )

        for _ in range(reps):  # reps>1 only for benchmarking (device-side loop)
            # Load yl and pre-scale by u0^6 (= 0.125 for Haar).
            yl_tmp = yh_pool.tile(
                [128, 16 * 64], fio, name="yl_tmp", tag="yl_tmp", bufs=1
            )
            yl_v = yl_t[:, :, :].rearrange("c (b r) w -> (c b) r w", b=4)
            if TUNE["probe"] != "compute":
                nc.sync.dma_start(
                    out=yl_tmp.rearrange("p (r w) -> p r w", w=64), in_=yl_v
                )
            else:
                nc.vector.memset(yl_tmp[0:1, 0:1], 0.0)
            if TUNE["probe"] != "dma":
                nc.scalar.mul(llA[:, :], yl_tmp[:, :], float(u0**6))

            # Level A (64x64 -> llB), one chunk.
            llA_v = llA.rearrange("p (r w) -> p r w", w=64)
            yhA = load_yh(yh2_v, 64, 16, 0, float(u0**6))
            emit_chunk(
                yhA, 0, 64, 16,
                llA_v[:, 0:16, :], interleave_dsts(llB, 64, 16, 0),
                float(u0**6),
            )

            # Levels B and C interleaved: each B chunk (8 in-rows -> 16 llC
            # rows per partition) feeds 4 C chunks immediately. With the PE
            # height pass there is no ACT prescale in the load chain, so
            # loads can optionally be batched 2 compute-chunks per DMA
            # (~1MB/channel at level C) without coarsening the pipeline;
            # consumers slice sub-chunks via rloc.
            llB_v = llB.rearrange("p (r w) -> p r w", w=128)
            RB, RC = 8, 4
            batch = 2 if TUNE["batch_loads"] else 1
            yhB = None
            for j in range(4):
                if j % batch == 0:
                    yhB = load_yh(yh1_v, 128, batch * RB, j * RB, float(u0**4))
                seg = segp.tile([128, 16 * 256], fio, name="seg", tag="seg")
                emit_chunk(
                    yhB, (j % batch) * RB, 128, RB,
                    llB_v[:, j * RB : (j + 1) * RB, :],
                    interleave_dsts(seg, 128, RB, 0),
                    float(u0**4),
                )
                seg_v = seg.rearrange("p (r w) -> p r w", w=256)
                yhC = None
                ot2 = None
                for i in range(4):
                    g0r = j * 16 + i * RC  # global C-level input row
                    if i % batch == 0:
                        yhC = load_yh(yh0_v, 256, batch * RC, g0r, float(u0**2))
                    if TUNE["batch_stores"]:
                        # one 4MB store per pair of chunks; the staging tile
                        # lives exactly 2 chunks (no slot-ring hogging).
                        if i % 2 == 0:
                            ot2 = outp.tile(
                                [128, 4 * RC * 512], fio, name="ot", tag="ot"
                            )
                        ot2_r = ot2.rearrange(
                            "p (h r ar w ac) -> p h ar ac r w",
                            h=2, ar=2, ac=2, w=256,
                        )
                        ot2_p = ot2.rearrange(
                            "p (h r ar w ac) -> p h ac r ar w",
                            h=2, ar=2, ac=2, w=256,
                        )
                        dsts = (
                            ot2_r[:, i % 2, 0, 0],
                            ot2_r[:, i % 2, 0, 1],
                            ot2_r[:, i % 2, 1, 0],
                            ot2_r[:, i % 2, 1, 1],
                            ot2_p[:, i % 2, 0],
                            ot2_p[:, i % 2, 1],
                        )
                        last = i % 2 == 1
                        emit_chunk(
                            yhC, (i % batch) * RC, 256, RC,
                            seg_v[:, i * RC : (i + 1) * RC, :],
                            dsts,
                            float(u0**2),
                            out_slice=(
                                out_v[:, 2 * (g0r - RC) : 2 * (g0r - RC) + 4 * RC, :]
                                if last
                                else None
                            ),
                            ot=(
                                ot2.rearrange("p (r w) -> p r w", w=512)
                                if last
                                else None
                            ),
                        )
                    else:
                        ot = outp.tile([128, 2 * RC * 512], fio, name="ot", tag="ot")
                        emit_chunk(
                            yhC, (i % batch) * RC, 256, RC,
                            seg_v[:, i * RC : (i + 1) * RC, :],
                            interleave_dsts(ot, 256, RC, 0),
                            float(u0**2),
                            out_slice=out_v[:, 2 * g0r : 2 * g0r + 2 * RC, :],
                            ot=ot.rearrange("p (r w) -> p r w", w=512),
                        )

    nc.compile()
    return nc


def _get_nc(u0, u1, v0, v1):
    key = (round(u0, 9), round(u1, 9), round(v0, 9), round(v1, 9))
    if key not in _cache:
        _cache[key] = _build_program(u0, u1, v0, v1)
    return _cache[key]


def _np_io():
    """numpy dtype for TUNE["io_dtype"] (bfloat16 needs ml_dtypes)."""
    if TUNE["io_dtype"] == "bfloat16":
        import ml_dtypes

        return np.dtype(ml_dtypes.bfloat16)
    return np.dtype(TUNE["io_dtype"])


def _run(inputs, trace=False, trace_kwargs=None):
    from concourse.bass_utils import run_bass_kernel_spmd

    np_io = _np_io()
    yl = np.ascontiguousarray(np.asarray(inputs["yl"]).astype(np_io))
    yh0 = np.ascontiguousarray(np.asarray(inputs["yh0"]).astype(np_io))
    yh1 = np.ascontiguousarray(np.asarray(inputs["yh1"]).astype(np_io))
    yh2 = np.ascontiguousarray(np.asarray(inputs["yh2"]).astype(np_io))
    g0 = np.asarray(inputs["g0"], dtype=np.float32)
    g1 = np.asarray(inputs["g1"], dtype=np.float32)

    u0, u1 = float(g0[0]), float(g0[1])
    v0, v1 = float(g1[0]), float(g1[1])

    nc = _get_nc(u0, u1, v0, v1)

    in_maps = [
        {"yl": yl[k], "yh0": yh0[k], "yh1": yh1[k], "yh2": yh2[k]}
        for k in range(N_CORES)
    ]
    kw = {}
    if trace:
        kw["trace"] = True
        if trace_kwargs:
            kw.update(trace_kwargs)
    res = run_bass_kernel_spmd(nc, in_maps, list(range(N_CORES)), **kw)
    out = np.stack([res.results[k]["out"] for k in range(N_CORES)], axis=0)
    return out.astype(np.float32, copy=False), res


def kernel(yl, yh0, yh1, yh2, g0, g1):
    out, _ = _run(
        {"yl": yl, "yh0": yh0, "yh1": yh1, "yh2": yh2, "g0": g0, "g1": g1}
    )
    return out

